# revision 4
# baseline (speedup 1.0000x reference)
"""Trainium2 Bass kernel for DirectionalFreqEmbed (per-token gather + grouped GEMM).

Token-parallel across 8 NeuronCores, one compiled program per core, tokens
greedy-balanced by chunk count. The host shards the inputs into per-core
operand panels: for each token the gathered x values are packed densely into
ceil((len+1)/128) chunks of 128 l-slots ([128, 64] bf16 panels, batch on the
free axis, plus a ones-slot that folds the bias into the GEMM), and the
per-token W rows are permuted to match ([128, 384] bf16 per chunk, zero rows
on padding). The device program is then a pure streaming block-GEMM: per
token one W-tile DMA and C_t accumulated bf16 matmuls into PSUM, a bf16
cast-copy, and a store. W is read exactly once at its true ragged size
(sum(lens) rows, ~97 MB chip-wide instead of the 283 MB dense padding).

kernel(**inputs) takes FULL unsharded inputs and returns the FULL output.
"""
import os
import sys

import ml_dtypes
import numpy as np

for _p in ("/opt/trn_rl_repo", "/root/.axon_site/_ro/trn_rl_repo"):
    if os.path.isdir(_p) and _p not in sys.path:
        sys.path.insert(0, _p)

try:  # the staged antenv lacks axon_hooks; inject a functional stand-in
    import antenv.axon_hooks  # noqa: F401
except ImportError:
    import types as _types

    _hooks = _types.ModuleType("antenv.axon_hooks")
    _hooks._hook = None
    _hooks.get_axon_ntff_profile_hook = lambda: _hooks._hook
    _hooks.set_axon_ntff_profile_hook = lambda h: setattr(_hooks, "_hook", h)
    sys.modules["antenv.axon_hooks"] = _hooks

import jax
import concourse.bass as bass  # noqa: F401
import concourse.tile as tile
from concourse import bacc, mybir

IMG, CIN, DIM, B = 64, 30, 384, 64
T, Lmax = 240, 1452
NCORES = 8

bf16 = mybir.dt.bfloat16
f32 = mybir.dt.float32

_cache = {}


def _assign_tokens(lens):
    """Greedy LPT balance of tokens across cores by chunk count."""
    C = np.ceil((lens.astype(np.int64) + 1) / 128).astype(np.int64)
    order = np.argsort(-C, kind="stable")
    loads = [0] * NCORES
    toks = [[] for _ in range(NCORES)]
    for t in order:
        k = min(range(NCORES), key=lambda k: (loads[k], len(toks[k])))
        toks[k].append(int(t))
        loads[k] += int(C[t])
    return [sorted(tk) for tk in toks], C


def _shard(x, W, bias, idx_a, idx_b, idx_c, lens):
    tok_lists, C = _assign_tokens(lens)
    in_maps, plans = [], []
    xbf = x.astype(np.float32)
    for k in range(NCORES):
        toks = tok_lists[k]
        totc = int(sum(C[t] for t in toks))
        x_core = np.zeros((128, totc * B), ml_dtypes.bfloat16)
        w_full_cols = []   # [128, 384] bf16 full chunks, concatenated
        w_last_rows = []   # packed partial last chunks, [K, 384]
        plan = []
        b0 = 0
        for t in toks:
            L = int(lens[t])
            c = int(C[t])
            klast = (L + 1) - (c - 1) * 128
            g = xbf[:, idx_c[t, :L], idx_a[t, :L], idx_b[t, :L]]  # [B, L]
            gp = np.zeros((c * 128, B), np.float32)
            gp[:L] = g.T
            gp[L] = 1.0  # ones-slot -> bias row
            x_core[:, b0 * B:(b0 + c) * B] = (
                gp.reshape(c, 128, B).transpose(1, 0, 2).reshape(128, c * B)
            ).astype(ml_dtypes.bfloat16)
            wp = np.zeros(((L + 1), DIM), np.float32)
            wp[:L] = W[t, :L]
            wp[L] = bias[t]
            wp = wp.astype(ml_dtypes.bfloat16)
            nfull = c if klast == 128 else c - 1
            if nfull:
                w_full_cols.append(
                    wp[:nfull * 128].reshape(nfull, 128, DIM)
                    .transpose(1, 0, 2).reshape(128, nfull * DIM))
            if klast < 128:
                w_last_rows.append(wp[(c - 1) * 128:])
            plan.append((c, klast))
            b0 += c
        w_full = (np.concatenate(w_full_cols, axis=1) if w_full_cols
                  else np.zeros((128, 0), ml_dtypes.bfloat16))
        w_last = (np.concatenate(w_last_rows, axis=0) if w_last_rows
                  else np.zeros((0, DIM), ml_dtypes.bfloat16))
        in_maps.append({"x_core": x_core, "w_full": w_full, "w_last": w_last})
        plans.append(plan)
    return in_maps, plans, tok_lists


def _build_program(plan):
    from contextlib import ExitStack

    tpc = len(plan)
    totc = sum(c for c, _ in plan)
    totf = sum((c if kl == 128 else c - 1) for c, kl in plan)
    totk = sum(kl for _, kl in plan if kl < 128)

    # group tokens into x tiles: first tile = 1 token (fast start), then ~24
    # chunks per tile so matmuls only depend on the x piece they read.
    groups, cur, acc = [], [], 0
    for j, (c, _) in enumerate(plan):
        cur.append(j)
        acc += c
        if acc >= 24 or (len(groups) == 0 and len(cur) == 1):
            groups.append(cur)
            cur, acc = [], 0
    if cur:
        groups.append(cur)

    nc = bacc.Bacc("TRN2", target_bir_lowering=False, debug=False, num_devices=1)
    x_core = nc.dram_tensor("x_core", [128, totc * B], bf16, kind="ExternalInput").ap()
    w_full = nc.dram_tensor("w_full", [128, max(totf, 1) * DIM], bf16,
                            kind="ExternalInput").ap()
    w_last = nc.dram_tensor("w_last", [max(totk, 1), DIM], bf16,
                            kind="ExternalInput").ap()
    y_core = nc.dram_tensor("y_core", [tpc, B, DIM], bf16, kind="ExternalOutput").ap()

    with tile.TileContext(nc) as tc, ExitStack() as ctx:
        x_pool = ctx.enter_context(tc.tile_pool(name="x", bufs=1))
        w_pool = ctx.enter_context(tc.tile_pool(name="w", bufs=8))
        ps_pool = ctx.enter_context(tc.tile_pool(name="ps", bufs=8, space="PSUM"))
        out_pool = ctx.enter_context(tc.tile_pool(name="o", bufs=4))

        # per-group x tiles on the scalar HWDGE ring (separate FIFO from the
        # W stream on sync) so each token only waits for its own x piece.
        x_tiles = {}
        goff = {}
        b0 = 0
        for gi, grp in enumerate(groups):
            gch = sum(plan[j][0] for j in grp)
            xt = x_pool.tile([128, gch * B], bf16, tag=f"x{gi}")
            nc.scalar.dma_start(xt[:], x_core[:, b0 * B:(b0 + gch) * B])
            for j in grp:
                x_tiles[j] = xt
                goff[j] = sum(plan[jj][0] for jj in grp[:grp.index(j)])
            b0 += gch

        f0 = 0   # column offset into w_full (chunks)
        k0 = 0   # row offset into w_last
        for j, (c, klast) in enumerate(plan):
            nfull = c if klast == 128 else c - 1
            w_tile = w_pool.tile([128, c * DIM], bf16, tag="w")
            if nfull:
                nc.sync.dma_start(w_tile[:, :nfull * DIM],
                                  w_full[:, f0 * DIM:(f0 + nfull) * DIM])
            if klast < 128:
                nc.sync.dma_start(w_tile[:klast, nfull * DIM:c * DIM],
                                  w_last[k0:k0 + klast, :])
            psum = ps_pool.tile([B, DIM], f32)
            xt, xo = x_tiles[j], goff[j]
            for ck in range(c):
                kk = 128 if ck < c - 1 else klast
                nc.tensor.matmul(
                    psum[:],
                    lhsT=xt[:kk, (xo + ck) * B:(xo + ck + 1) * B],
                    rhs=w_tile[:kk, ck * DIM:(ck + 1) * DIM],
                    start=(ck == 0),
                    stop=(ck == c - 1),
                )
            o_tile = out_pool.tile([B, DIM], bf16)
            nc.vector.tensor_copy(o_tile[:], psum[:])
            nc.scalar.dma_start(y_core[j], o_tile[:])
            f0 += nfull
            k0 += klast if klast < 128 else 0

    nc.compile()
    return nc


def _run_per_core(ncs, in_maps):
    """Per-device execution of 8 distinct single-core programs (adapted from
    bass2jax.run_bass_via_pjrt's single-core path)."""
    from concurrent.futures import ThreadPoolExecutor

    from concourse import mybir as mb
    from concourse.bass2jax import _bass_exec_p, install_neuronx_cc_hook

    install_neuronx_cc_hook()
    devices = jax.devices()[:8]

    def launch(k):
        nc = ncs[k]
        in_names, out_names, out_avals, zero_outs = [], [], [], []
        for alloc in nc.m.functions[0].allocations:
            if not isinstance(alloc, mb.MemoryLocationSet):
                continue
            name = alloc.memorylocations[0].name
            if alloc.kind == "ExternalInput":
                in_names.append(name)
            elif alloc.kind == "ExternalOutput":
                shape = tuple(alloc.tensor_shape)
                dtype = mb.dt.np(alloc.dtype)
                out_names.append(name)
                out_avals.append(jax.core.ShapedArray(shape, dtype))
                zero_outs.append(np.zeros(shape, dtype))
        n_params = len(in_names)
        all_names = tuple(in_names + out_names)
        donate = tuple(range(n_params, n_params + len(out_names)))

        def _body(*args):
            outs = _bass_exec_p.bind(
                *args,
                out_avals=tuple(out_avals),
                in_names=all_names,
                out_names=tuple(out_names),
                lowering_input_output_aliases=(),
                sim_require_finite=True,
                sim_require_nnan=True,
                nc=nc,
            )
            return tuple(outs)

        dev = devices[k]
        extras = {}
        for alloc in nc.m.functions[0].allocations:
            if (isinstance(alloc, mb.MemoryLocationSet)
                    and alloc.kind == "ExternalInput"):
                name = alloc.memorylocations[0].name
                if name not in in_maps[k]:
                    extras[name] = np.full(
                        tuple(alloc.tensor_shape), k, mb.dt.np(alloc.dtype))
        args = [jax.device_put(np.asarray(in_maps[k].get(n, extras.get(n))), dev)
                for n in in_names]
        args += [jax.device_put(z, dev) for z in zero_outs]
        out_arrs = jax.jit(_body, donate_argnums=donate, keep_unused=True)(*args)
        return out_names, out_arrs

    with ThreadPoolExecutor(max_workers=8) as ex:
        futs = [ex.submit(launch, k) for k in range(8)]
        handles = [f.result() for f in futs]
    return [
        {name: np.asarray(arr) for name, arr in zip(names, arrs)}
        for names, arrs in handles
    ]


LAST_RESULTS = None


def kernel(x, W, bias, idx_a, idx_b, idx_c, lens):
    global LAST_RESULTS
    x = np.asarray(x, np.float32)
    W = np.asarray(W, np.float32)
    bias = np.asarray(bias, np.float32)
    idx_a = np.asarray(idx_a, np.int32)
    idx_b = np.asarray(idx_b, np.int32)
    idx_c = np.asarray(idx_c, np.int32)
    lens = np.asarray(lens, np.int32)
    assert x.shape == (B, CIN, IMG, IMG) and W.shape == (T, Lmax, DIM)

    in_maps, plans, tok_lists = _shard(x, W, bias, idx_a, idx_b, idx_c, lens)
    if "ncs" not in _cache:
        _cache["ncs"] = [_build_program(plans[k]) for k in range(NCORES)]
    ncs = _cache["ncs"]

    hook = None
    trace = os.environ.get("BASS_TRACE") and not os.environ.get("BASS_NEVER_TRACE")
    if trace:
        from antenv.axon_hooks import get_axon_ntff_profile_hook

        hook = get_axon_ntff_profile_hook()
    if hook is not None:
        tmpdir = os.environ.get("KERNEL_TRACE_TMPDIR") or "/tmp/kernel_trace"
        os.makedirs(tmpdir, exist_ok=True)
        with hook(tmpdir, [0]):
            results = _run_per_core(ncs, in_maps)
        LAST_RESULTS = ("ntff", tmpdir, ncs[0])
    else:
        results = _run_per_core(ncs, in_maps)
        LAST_RESULTS = None

    y = np.empty((B, T, DIM), np.float32)
    for k in range(NCORES):
        y[:, tok_lists[k], :] = results[k]["y_core"].transpose(1, 0, 2).astype(
            np.float32)
    return y


# revision 6
# speedup vs baseline: 1.3941x; 1.3941x over previous
"""Trainium2 Bass kernel for DirectionalFreqEmbed (per-token gather + grouped GEMM).

Token-parallel across 8 NeuronCores, one compiled program per core, tokens
greedy-balanced by chunk count. The host shards the inputs into per-core
operand panels: for each token the gathered x values are packed densely into
ceil((len+1)/128) chunks of 128 l-slots ([128, 64] bf16 panels, batch on the
free axis, plus a ones-slot that folds the bias into the GEMM), and the
per-token W rows are permuted to match ([128, 384] bf16 per chunk, zero rows
on padding). The device program is then a pure streaming block-GEMM: per
token one W-tile DMA and C_t accumulated bf16 matmuls into PSUM, a bf16
cast-copy, and a store. W is read exactly once at its true ragged size
(sum(lens) rows, ~97 MB chip-wide instead of the 283 MB dense padding).

kernel(**inputs) takes FULL unsharded inputs and returns the FULL output.
"""
import os
import sys

import ml_dtypes
import numpy as np

for _p in ("/opt/trn_rl_repo", "/root/.axon_site/_ro/trn_rl_repo"):
    if os.path.isdir(_p) and _p not in sys.path:
        sys.path.insert(0, _p)

try:  # the staged antenv lacks axon_hooks; inject a functional stand-in
    import antenv.axon_hooks  # noqa: F401
except ImportError:
    import types as _types

    _hooks = _types.ModuleType("antenv.axon_hooks")
    _hooks._hook = None
    _hooks.get_axon_ntff_profile_hook = lambda: _hooks._hook
    _hooks.set_axon_ntff_profile_hook = lambda h: setattr(_hooks, "_hook", h)
    sys.modules["antenv.axon_hooks"] = _hooks

import jax
import concourse.bass as bass  # noqa: F401
import concourse.tile as tile
from concourse import bacc, mybir

IMG, CIN, DIM, B = 64, 30, 384, 64
T, Lmax = 240, 1452
NCORES = 8

bf16 = mybir.dt.bfloat16
f32 = mybir.dt.float32

_cache = {}


def _assign_tokens(lens):
    """Greedy LPT balance of tokens across cores by chunk count."""
    C = np.ceil((lens.astype(np.int64) + 1) / 128).astype(np.int64)
    order = np.argsort(-C, kind="stable")
    loads = [0] * NCORES
    toks = [[] for _ in range(NCORES)]
    for t in order:
        k = min(range(NCORES), key=lambda k: (loads[k], len(toks[k])))
        toks[k].append(int(t))
        loads[k] += int(C[t])
    return [sorted(tk) for tk in toks], C


def _shard(x, W, bias, idx_a, idx_b, idx_c, lens):
    tok_lists, C = _assign_tokens(lens)
    in_maps, plans = [], []
    xbf = x.astype(np.float32)
    for k in range(NCORES):
        toks = tok_lists[k]
        totc = int(sum(C[t] for t in toks))
        x_core = np.zeros((128, totc * B), ml_dtypes.bfloat16)
        w_core = np.zeros((128, totc * DIM), ml_dtypes.bfloat16)
        plan = []
        b0 = 0
        for t in toks:
            L = int(lens[t])
            c = int(C[t])
            g = xbf[:, idx_c[t, :L], idx_a[t, :L], idx_b[t, :L]]  # [B, L]
            gp = np.zeros((c * 128, B), np.float32)
            gp[:L] = g.T
            gp[L] = 1.0  # ones-slot -> bias row
            x_core[:, b0 * B:(b0 + c) * B] = (
                gp.reshape(c, 128, B).transpose(1, 0, 2).reshape(128, c * B)
            ).astype(ml_dtypes.bfloat16)
            wp = np.zeros((c * 128, DIM), np.float32)
            wp[:L] = W[t, :L]
            wp[L] = bias[t]
            w_core[:, b0 * DIM:(b0 + c) * DIM] = (
                wp.reshape(c, 128, DIM).transpose(1, 0, 2).reshape(128, c * DIM)
            ).astype(ml_dtypes.bfloat16)
            plan.append(c)
            b0 += c
        in_maps.append({"x_core": x_core, "w_core": w_core})
        plans.append(plan)
    return in_maps, plans, tok_lists


def _build_program(plan):
    from contextlib import ExitStack

    tpc = len(plan)
    totc = sum(plan)

    # group tokens into x tiles: first tile = 1 token (fast start), then ~24
    # chunks per tile so matmuls only depend on the x piece they read.
    groups, cur, acc = [], [], 0
    for j, c in enumerate(plan):
        cur.append(j)
        acc += c
        if acc >= 24 or (len(groups) == 0 and len(cur) == 1):
            groups.append(cur)
            cur, acc = [], 0
    if cur:
        groups.append(cur)

    nc = bacc.Bacc("TRN2", target_bir_lowering=False, debug=False, num_devices=1)
    x_core = nc.dram_tensor("x_core", [128, totc * B], bf16, kind="ExternalInput").ap()
    w_core = nc.dram_tensor("w_core", [128, totc * DIM], bf16,
                            kind="ExternalInput").ap()
    y_core = nc.dram_tensor("y_core", [tpc, B, DIM], bf16, kind="ExternalOutput").ap()

    with tile.TileContext(nc) as tc, ExitStack() as ctx:
        x_pool = ctx.enter_context(tc.tile_pool(name="x", bufs=1))
        w_pool = ctx.enter_context(tc.tile_pool(name="w", bufs=12))
        ps_pool = ctx.enter_context(tc.tile_pool(name="ps", bufs=7, space="PSUM"))
        dum_pool = ctx.enter_context(tc.tile_pool(name="du", bufs=1, space="PSUM"))
        out_pool = ctx.enter_context(tc.tile_pool(name="o", bufs=4))

        # per-group x tiles on the scalar HWDGE ring (separate FIFO from the
        # W stream on sync) so each token only waits for its own x piece.
        x_tiles = {}
        goff = {}
        b0 = 0
        for gi, grp in enumerate(groups):
            gch = sum(plan[j] for j in grp)
            xt = x_pool.tile([128, gch * B], bf16, tag=f"x{gi}")
            nc.scalar.dma_start(xt[:], x_core[:, b0 * B:(b0 + gch) * B])
            o = 0
            for j in grp:
                x_tiles[j] = xt
                goff[j] = o
                o += plan[j]
            b0 += gch

        # scratch operand for PE warm-keeper matmuls (keeps the HAM clock
        # gate at 8/8 across DMA-starved gaps; results are never read).
        dummy_sb = x_tiles[0]
        dummy_ps = dum_pool.tile([B, B], f32)

        b0 = 0
        for j, c in enumerate(plan):
            w_tile = w_pool.tile([128, c * DIM], bf16, tag="w")
            nc.sync.dma_start(w_tile[:], w_core[:, b0 * DIM:(b0 + c) * DIM])
            psum = ps_pool.tile([B, DIM], f32)
            xt, xo = x_tiles[j], goff[j]
            for ck in range(c):
                nc.tensor.matmul(
                    psum[:],
                    lhsT=xt[:, (xo + ck) * B:(xo + ck + 1) * B],
                    rhs=w_tile[:, ck * DIM:(ck + 1) * DIM],
                    start=(ck == 0),
                    stop=(ck == c - 1),
                )
            for _ in range(2):
                nc.tensor.matmul(dummy_ps[:], lhsT=dummy_sb[:, :B],
                                 rhs=dummy_sb[:, :B], start=True, stop=True)
            o_tile = out_pool.tile([B, DIM], bf16)
            nc.vector.tensor_copy(o_tile[:], psum[:])
            nc.scalar.dma_start(y_core[j], o_tile[:])
            b0 += c

    nc.compile()
    return nc


def _run_per_core(ncs, in_maps):
    """Per-device execution of 8 distinct single-core programs (adapted from
    bass2jax.run_bass_via_pjrt's single-core path)."""
    from concurrent.futures import ThreadPoolExecutor

    from concourse import mybir as mb
    from concourse.bass2jax import _bass_exec_p, install_neuronx_cc_hook

    install_neuronx_cc_hook()
    devices = jax.devices()[:8]

    def launch(k):
        nc = ncs[k]
        in_names, out_names, out_avals, zero_outs = [], [], [], []
        for alloc in nc.m.functions[0].allocations:
            if not isinstance(alloc, mb.MemoryLocationSet):
                continue
            name = alloc.memorylocations[0].name
            if alloc.kind == "ExternalInput":
                in_names.append(name)
            elif alloc.kind == "ExternalOutput":
                shape = tuple(alloc.tensor_shape)
                dtype = mb.dt.np(alloc.dtype)
                out_names.append(name)
                out_avals.append(jax.core.ShapedArray(shape, dtype))
                zero_outs.append(np.zeros(shape, dtype))
        n_params = len(in_names)
        all_names = tuple(in_names + out_names)
        donate = tuple(range(n_params, n_params + len(out_names)))

        def _body(*args):
            outs = _bass_exec_p.bind(
                *args,
                out_avals=tuple(out_avals),
                in_names=all_names,
                out_names=tuple(out_names),
                lowering_input_output_aliases=(),
                sim_require_finite=True,
                sim_require_nnan=True,
                nc=nc,
            )
            return tuple(outs)

        dev = devices[k]
        extras = {}
        for alloc in nc.m.functions[0].allocations:
            if (isinstance(alloc, mb.MemoryLocationSet)
                    and alloc.kind == "ExternalInput"):
                name = alloc.memorylocations[0].name
                if name not in in_maps[k]:
                    extras[name] = np.full(
                        tuple(alloc.tensor_shape), k, mb.dt.np(alloc.dtype))
        args = [jax.device_put(np.asarray(in_maps[k].get(n, extras.get(n))), dev)
                for n in in_names]
        args += [jax.device_put(z, dev) for z in zero_outs]
        out_arrs = jax.jit(_body, donate_argnums=donate, keep_unused=True)(*args)
        return out_names, out_arrs

    with ThreadPoolExecutor(max_workers=8) as ex:
        futs = [ex.submit(launch, k) for k in range(8)]
        handles = [f.result() for f in futs]
    return [
        {name: np.asarray(arr) for name, arr in zip(names, arrs)}
        for names, arrs in handles
    ]


LAST_RESULTS = None


def kernel(x, W, bias, idx_a, idx_b, idx_c, lens):
    global LAST_RESULTS
    x = np.asarray(x, np.float32)
    W = np.asarray(W, np.float32)
    bias = np.asarray(bias, np.float32)
    idx_a = np.asarray(idx_a, np.int32)
    idx_b = np.asarray(idx_b, np.int32)
    idx_c = np.asarray(idx_c, np.int32)
    lens = np.asarray(lens, np.int32)
    assert x.shape == (B, CIN, IMG, IMG) and W.shape == (T, Lmax, DIM)

    in_maps, plans, tok_lists = _shard(x, W, bias, idx_a, idx_b, idx_c, lens)
    if "ncs" not in _cache:
        _cache["ncs"] = [_build_program(plans[k]) for k in range(NCORES)]
    ncs = _cache["ncs"]

    hook = None
    trace = os.environ.get("BASS_TRACE") and not os.environ.get("BASS_NEVER_TRACE")
    if trace:
        from antenv.axon_hooks import get_axon_ntff_profile_hook

        hook = get_axon_ntff_profile_hook()
    if hook is not None:
        tmpdir = os.environ.get("KERNEL_TRACE_TMPDIR") or "/tmp/kernel_trace"
        os.makedirs(tmpdir, exist_ok=True)
        with hook(tmpdir, [0]):
            results = _run_per_core(ncs, in_maps)
        LAST_RESULTS = ("ntff", tmpdir, ncs[0])
    else:
        results = _run_per_core(ncs, in_maps)
        LAST_RESULTS = None

    y = np.empty((B, T, DIM), np.float32)
    for k in range(NCORES):
        y[:, tok_lists[k], :] = results[k]["y_core"].transpose(1, 0, 2).astype(
            np.float32)
    return y


# revision 8
# speedup vs baseline: 1.4421x; 1.0345x over previous
"""Trainium2 Bass kernel for DirectionalFreqEmbed (per-token gather + grouped GEMM).

Token-parallel across 8 NeuronCores, one compiled program per core, tokens
greedy-balanced by chunk count. The host shards the inputs into per-core
operand panels: for each token the gathered x values are packed densely into
ceil((len+1)/128) chunks of 128 l-slots ([128, 64] bf16 panels, batch on the
free axis, plus a ones-slot that folds the bias into the GEMM), and the
per-token W rows are permuted to match ([128, 384] bf16 per chunk, zero rows
on padding). The device program is then a pure streaming block-GEMM: per
token one W-tile DMA and C_t accumulated bf16 matmuls into PSUM, a bf16
cast-copy, and a store. W is read exactly once at its true ragged size
(sum(lens) rows, ~97 MB chip-wide instead of the 283 MB dense padding).

kernel(**inputs) takes FULL unsharded inputs and returns the FULL output.
"""
import os
import sys

import ml_dtypes
import numpy as np

for _p in ("/opt/trn_rl_repo", "/root/.axon_site/_ro/trn_rl_repo"):
    if os.path.isdir(_p) and _p not in sys.path:
        sys.path.insert(0, _p)

try:  # the staged antenv lacks axon_hooks; inject a functional stand-in
    import antenv.axon_hooks  # noqa: F401
except ImportError:
    import types as _types

    _hooks = _types.ModuleType("antenv.axon_hooks")
    _hooks._hook = None
    _hooks.get_axon_ntff_profile_hook = lambda: _hooks._hook
    _hooks.set_axon_ntff_profile_hook = lambda h: setattr(_hooks, "_hook", h)
    sys.modules["antenv.axon_hooks"] = _hooks

import jax
import concourse.bass as bass  # noqa: F401
import concourse.tile as tile
from concourse import bacc, mybir

IMG, CIN, DIM, B = 64, 30, 384, 64
T, Lmax = 240, 1452
NCORES = 8

bf16 = mybir.dt.bfloat16
f32 = mybir.dt.float32

_cache = {}


def _assign_tokens(lens):
    """Greedy LPT balance of tokens across cores by chunk count."""
    C = np.ceil((lens.astype(np.int64) + 1) / 128).astype(np.int64)
    order = np.argsort(-C, kind="stable")
    loads = [0] * NCORES
    toks = [[] for _ in range(NCORES)]
    for t in order:
        k = min(range(NCORES), key=lambda k: (loads[k], len(toks[k])))
        toks[k].append(int(t))
        loads[k] += int(C[t])
    # per-core order: 3 smallest first (fast pipeline start), then the rest
    # descending so the final token is small (short post-DMA tail chain).
    out = []
    for tk in toks:
        s = sorted(tk, key=lambda t: int(C[t]))
        out.append(s[:3] + s[3:][::-1])
    return out, C


def _shard(x, W, bias, idx_a, idx_b, idx_c, lens):
    tok_lists, C = _assign_tokens(lens)
    in_maps, plans = [], []
    xbf = x.astype(np.float32)
    for k in range(NCORES):
        toks = tok_lists[k]
        totc = int(sum(C[t] for t in toks))
        x_core = np.zeros((128, totc * B), ml_dtypes.bfloat16)
        w_core = np.zeros((128, totc * DIM), ml_dtypes.bfloat16)
        plan = []
        b0 = 0
        for t in toks:
            L = int(lens[t])
            c = int(C[t])
            g = xbf[:, idx_c[t, :L], idx_a[t, :L], idx_b[t, :L]]  # [B, L]
            gp = np.zeros((c * 128, B), np.float32)
            gp[:L] = g.T
            gp[L] = 1.0  # ones-slot -> bias row
            x_core[:, b0 * B:(b0 + c) * B] = (
                gp.reshape(c, 128, B).transpose(1, 0, 2).reshape(128, c * B)
            ).astype(ml_dtypes.bfloat16)
            wp = np.zeros((c * 128, DIM), np.float32)
            wp[:L] = W[t, :L]
            wp[L] = bias[t]
            w_core[:, b0 * DIM:(b0 + c) * DIM] = (
                wp.reshape(c, 128, DIM).transpose(1, 0, 2).reshape(128, c * DIM)
            ).astype(ml_dtypes.bfloat16)
            plan.append(c)
            b0 += c
        in_maps.append({"x_core": x_core, "w_core": w_core})
        plans.append(plan)
    return in_maps, plans, tok_lists


def _build_program(plan):
    from contextlib import ExitStack

    tpc = len(plan)
    totc = sum(plan)

    # group tokens into x tiles: first tile = 1 token (fast start), then ~24
    # chunks per tile so matmuls only depend on the x piece they read.
    groups, cur, acc = [], [], 0
    for j, c in enumerate(plan):
        cur.append(j)
        acc += c
        if acc >= 24 or (len(groups) == 0 and len(cur) == 1):
            groups.append(cur)
            cur, acc = [], 0
    if cur:
        groups.append(cur)

    nc = bacc.Bacc("TRN2", target_bir_lowering=False, debug=False, num_devices=1)
    x_core = nc.dram_tensor("x_core", [128, totc * B], bf16, kind="ExternalInput").ap()
    w_core = nc.dram_tensor("w_core", [128, totc * DIM], bf16,
                            kind="ExternalInput").ap()
    y_core = nc.dram_tensor("y_core", [tpc, B, DIM], bf16, kind="ExternalOutput").ap()

    with tile.TileContext(nc) as tc, ExitStack() as ctx:
        x_pool = ctx.enter_context(tc.tile_pool(name="x", bufs=1))
        w_pool = ctx.enter_context(tc.tile_pool(name="w", bufs=12))
        ps_pool = ctx.enter_context(tc.tile_pool(name="ps", bufs=7, space="PSUM"))
        dum_pool = ctx.enter_context(tc.tile_pool(name="du", bufs=1, space="PSUM"))
        out_pool = ctx.enter_context(tc.tile_pool(name="o", bufs=4))

        # x tiles load just-in-time, interleaved into the scalar ring; W tiles
        # alternate between the two HWDGE rings (sync=SP, scalar=ACT) so
        # descriptor generation runs in parallel.
        tok_group = {}
        goff = {}
        gstart = []
        o = 0
        for gi, grp in enumerate(groups):
            gstart.append(o)
            for j in grp:
                tok_group[j] = gi
                goff[j] = o
                o += plan[j]
        x_tiles = [None] * len(groups)

        def emit_xgroup(gi):
            grp = groups[gi]
            gch = sum(plan[j] for j in grp)
            xt = x_pool.tile([128, gch * B], bf16, tag=f"x{gi}")
            nc.scalar.dma_start(
                xt[:], x_core[:, gstart[gi] * B:(gstart[gi] + gch) * B])
            x_tiles[gi] = xt

        emit_xgroup(0)
        xg_next = 1

        # scratch operand for PE warm-keeper matmuls (keeps the HAM clock
        # gate at 8/8 across DMA-starved gaps; results are never read).
        dummy_sb = x_tiles[0]
        dummy_ps = dum_pool.tile([B, B], f32)

        b0 = 0
        for j, c in enumerate(plan):
            want = tok_group[min(j + 4, tpc - 1)]
            while xg_next <= want:
                emit_xgroup(xg_next)
                xg_next += 1
            w_tile = w_pool.tile([128, c * DIM], bf16, tag="w")
            ring = nc.sync if j % 2 == 0 else nc.scalar
            ring.dma_start(w_tile[:], w_core[:, b0 * DIM:(b0 + c) * DIM])
            psum = ps_pool.tile([B, DIM], f32)
            xt, xo = x_tiles[tok_group[j]], goff[j] - gstart[tok_group[j]]
            for ck in range(c):
                nc.tensor.matmul(
                    psum[:],
                    lhsT=xt[:, (xo + ck) * B:(xo + ck + 1) * B],
                    rhs=w_tile[:, ck * DIM:(ck + 1) * DIM],
                    start=(ck == 0),
                    stop=(ck == c - 1),
                )
            for _ in range(2):
                nc.tensor.matmul(dummy_ps[:], lhsT=dummy_sb[:, :B],
                                 rhs=dummy_sb[:, :B], start=True, stop=True)
            o_tile = out_pool.tile([B, DIM], bf16)
            nc.vector.tensor_copy(o_tile[:], psum[:])
            oring = nc.scalar if j % 2 == 0 else nc.sync
            oring.dma_start(y_core[j], o_tile[:])
            b0 += c

    nc.compile()
    return nc


def _run_per_core(ncs, in_maps):
    """Per-device execution of 8 distinct single-core programs (adapted from
    bass2jax.run_bass_via_pjrt's single-core path)."""
    from concurrent.futures import ThreadPoolExecutor

    from concourse import mybir as mb
    from concourse.bass2jax import _bass_exec_p, install_neuronx_cc_hook

    install_neuronx_cc_hook()
    devices = jax.devices()[:8]

    def launch(k):
        nc = ncs[k]
        in_names, out_names, out_avals, zero_outs = [], [], [], []
        for alloc in nc.m.functions[0].allocations:
            if not isinstance(alloc, mb.MemoryLocationSet):
                continue
            name = alloc.memorylocations[0].name
            if alloc.kind == "ExternalInput":
                in_names.append(name)
            elif alloc.kind == "ExternalOutput":
                shape = tuple(alloc.tensor_shape)
                dtype = mb.dt.np(alloc.dtype)
                out_names.append(name)
                out_avals.append(jax.core.ShapedArray(shape, dtype))
                zero_outs.append(np.zeros(shape, dtype))
        n_params = len(in_names)
        all_names = tuple(in_names + out_names)
        donate = tuple(range(n_params, n_params + len(out_names)))

        def _body(*args):
            outs = _bass_exec_p.bind(
                *args,
                out_avals=tuple(out_avals),
                in_names=all_names,
                out_names=tuple(out_names),
                lowering_input_output_aliases=(),
                sim_require_finite=True,
                sim_require_nnan=True,
                nc=nc,
            )
            return tuple(outs)

        dev = devices[k]
        extras = {}
        for alloc in nc.m.functions[0].allocations:
            if (isinstance(alloc, mb.MemoryLocationSet)
                    and alloc.kind == "ExternalInput"):
                name = alloc.memorylocations[0].name
                if name not in in_maps[k]:
                    extras[name] = np.full(
                        tuple(alloc.tensor_shape), k, mb.dt.np(alloc.dtype))
        args = [jax.device_put(np.asarray(in_maps[k].get(n, extras.get(n))), dev)
                for n in in_names]
        args += [jax.device_put(z, dev) for z in zero_outs]
        out_arrs = jax.jit(_body, donate_argnums=donate, keep_unused=True)(*args)
        return out_names, out_arrs

    with ThreadPoolExecutor(max_workers=8) as ex:
        futs = [ex.submit(launch, k) for k in range(8)]
        handles = [f.result() for f in futs]
    return [
        {name: np.asarray(arr) for name, arr in zip(names, arrs)}
        for names, arrs in handles
    ]


LAST_RESULTS = None


def kernel(x, W, bias, idx_a, idx_b, idx_c, lens):
    global LAST_RESULTS
    x = np.asarray(x, np.float32)
    W = np.asarray(W, np.float32)
    bias = np.asarray(bias, np.float32)
    idx_a = np.asarray(idx_a, np.int32)
    idx_b = np.asarray(idx_b, np.int32)
    idx_c = np.asarray(idx_c, np.int32)
    lens = np.asarray(lens, np.int32)
    assert x.shape == (B, CIN, IMG, IMG) and W.shape == (T, Lmax, DIM)

    in_maps, plans, tok_lists = _shard(x, W, bias, idx_a, idx_b, idx_c, lens)
    if "ncs" not in _cache:
        _cache["ncs"] = [_build_program(plans[k]) for k in range(NCORES)]
    ncs = _cache["ncs"]

    hook = None
    trace = os.environ.get("BASS_TRACE") and not os.environ.get("BASS_NEVER_TRACE")
    if trace:
        from antenv.axon_hooks import get_axon_ntff_profile_hook

        hook = get_axon_ntff_profile_hook()
    if hook is not None:
        tmpdir = os.environ.get("KERNEL_TRACE_TMPDIR") or "/tmp/kernel_trace"
        os.makedirs(tmpdir, exist_ok=True)
        with hook(tmpdir, [0]):
            results = _run_per_core(ncs, in_maps)
        LAST_RESULTS = ("ntff", tmpdir, ncs[0])
    else:
        results = _run_per_core(ncs, in_maps)
        LAST_RESULTS = None

    y = np.empty((B, T, DIM), np.float32)
    for k in range(NCORES):
        y[:, tok_lists[k], :] = results[k]["y_core"].transpose(1, 0, 2).astype(
            np.float32)
    return y


# revision 13
# speedup vs baseline: 1.4506x; 1.0059x over previous
"""Trainium2 Bass kernel for DirectionalFreqEmbed (per-token gather + grouped GEMM).

Token-parallel across 8 NeuronCores, one compiled program per core, tokens
greedy-balanced by chunk count. The host shards the inputs into per-core
operand panels: for each token the gathered x values are packed densely into
ceil((len+1)/128) chunks of 128 l-slots ([128, 64] bf16 panels, batch on the
free axis, plus a ones-slot that folds the bias into the GEMM), and the
per-token W rows are permuted to match ([128, 384] bf16 per chunk, zero rows
on padding). The device program is then a pure streaming block-GEMM: per
token one W-tile DMA and C_t accumulated bf16 matmuls into PSUM, a bf16
cast-copy, and a store. W is read exactly once at its true ragged size
(sum(lens) rows, ~97 MB chip-wide instead of the 283 MB dense padding).

kernel(**inputs) takes FULL unsharded inputs and returns the FULL output.
"""
import os
import sys

import ml_dtypes
import numpy as np

for _p in ("/opt/trn_rl_repo", "/root/.axon_site/_ro/trn_rl_repo"):
    if os.path.isdir(_p) and _p not in sys.path:
        sys.path.insert(0, _p)

try:  # the staged antenv lacks axon_hooks; inject a functional stand-in
    import antenv.axon_hooks  # noqa: F401
except ImportError:
    import types as _types

    _hooks = _types.ModuleType("antenv.axon_hooks")
    _hooks._hook = None
    _hooks.get_axon_ntff_profile_hook = lambda: _hooks._hook
    _hooks.set_axon_ntff_profile_hook = lambda h: setattr(_hooks, "_hook", h)
    sys.modules["antenv.axon_hooks"] = _hooks

import jax
import concourse.bass as bass  # noqa: F401
import concourse.tile as tile
from concourse import bacc, mybir

IMG, CIN, DIM, B = 64, 30, 384, 64
T, Lmax = 240, 1452
NCORES = 8

bf16 = mybir.dt.bfloat16
f32 = mybir.dt.float32

_cache = {}


def _assign_tokens(lens):
    """Greedy LPT balance of tokens across cores by chunk count."""
    C = np.ceil((lens.astype(np.int64) + 1) / 128).astype(np.int64)
    order = np.argsort(-C, kind="stable")
    loads = [0] * NCORES
    toks = [[] for _ in range(NCORES)]
    for t in order:
        k = min(range(NCORES), key=lambda k: (loads[k], len(toks[k])))
        toks[k].append(int(t))
        loads[k] += int(C[t])
    # per-core order: 3 smallest first (fast pipeline start), then the rest
    # descending so the final token is small (short post-DMA tail chain).
    out = []
    for tk in toks:
        s = sorted(tk, key=lambda t: int(C[t]))
        out.append(s[:3] + s[3:][::-1])
    return out, C


def _shard(x, W, bias, idx_a, idx_b, idx_c, lens):
    """Per-core slot stream: each token padded to whole 128-slot chunks
    (PE base-partition rule requires chunk starts at partition 0), packed
    into [128, *] panels for full SBUF residency."""
    tok_lists, C = _assign_tokens(lens)
    in_maps, plans = [], []
    xbf = x.astype(np.float32)
    for k in range(NCORES):
        toks = tok_lists[k]
        nchunks = int(sum(C[t] for t in toks))
        xg = np.zeros((nchunks * 128, B), np.float32)
        wg = np.zeros((nchunks * 128, DIM), np.float32)
        plan = []
        s0 = 0
        for t in toks:
            L = int(lens[t])
            c = int(C[t])
            g = xbf[:, idx_c[t, :L], idx_a[t, :L], idx_b[t, :L]]  # [B, L]
            xg[s0:s0 + L] = g.T
            xg[s0 + L] = 1.0  # ones-slot -> bias row
            wg[s0:s0 + L] = W[t, :L]
            wg[s0 + L] = bias[t]
            plan.append(c)
            s0 += c * 128
        x_core = np.ascontiguousarray(
            xg.reshape(nchunks, 128, B).transpose(1, 0, 2).reshape(128, -1)
        ).astype(ml_dtypes.bfloat16)
        w_core = np.ascontiguousarray(
            wg.reshape(nchunks, 128, DIM).transpose(1, 0, 2).reshape(128, -1)
        ).astype(ml_dtypes.bfloat16)
        in_maps.append({"x_core": x_core, "w_core": w_core})
        plans.append(plan)
    return in_maps, plans, tok_lists


def _build_program(plan):
    from contextlib import ExitStack

    tpc = len(plan)
    nchunks = sum(plan)

    # W/x panels are fully SBUF-resident, loaded in a few large growing
    # pieces (piece boundaries on token boundaries) so PE can start early
    # while DMA streams at near line rate.
    tok_chunk0 = []
    c0 = 0
    for c in plan:
        tok_chunk0.append(c0)
        c0 += c
    pieces, cur0, acc, want = [], 0, 0, 4
    for j, c in enumerate(plan):
        acc += c
        if acc >= want or j == tpc - 1:
            pieces.append((cur0, acc))
            cur0 += acc
            acc = 0
            want = min(want * 2, 24)
    piece_of_chunk = {}
    for pi, (p0, n) in enumerate(pieces):
        for cc in range(p0, p0 + n):
            piece_of_chunk[cc] = pi

    nc = bacc.Bacc("TRN2", target_bir_lowering=False, debug=False, num_devices=1)
    x_core = nc.dram_tensor("x_core", [128, nchunks * B], bf16,
                            kind="ExternalInput").ap()
    w_core = nc.dram_tensor("w_core", [128, nchunks * DIM], bf16,
                            kind="ExternalInput").ap()
    y_core = nc.dram_tensor("y_core", [tpc, B, DIM], bf16, kind="ExternalOutput").ap()

    with tile.TileContext(nc) as tc, ExitStack() as ctx:
        sb_pool = ctx.enter_context(tc.tile_pool(name="sb", bufs=1))
        ps_pool = ctx.enter_context(tc.tile_pool(name="ps", bufs=8, space="PSUM"))
        out_pool = ctx.enter_context(tc.tile_pool(name="o", bufs=4))

        x_tiles, w_tiles = [], []
        for pi, (p0, n) in enumerate(pieces):
            xt = sb_pool.tile([128, n * B], bf16, tag=f"x{pi}")
            nc.scalar.dma_start(xt[:], x_core[:, p0 * B:(p0 + n) * B])
            wt = sb_pool.tile([128, n * DIM], bf16, tag=f"w{pi}")
            nc.sync.dma_start(wt[:], w_core[:, p0 * DIM:(p0 + n) * DIM])
            x_tiles.append(xt)
            w_tiles.append(wt)

        for j, c in enumerate(plan):
            psum = ps_pool.tile([B, DIM], f32)
            for ck in range(c):
                cc = tok_chunk0[j] + ck
                pi = piece_of_chunk[cc]
                lc = cc - pieces[pi][0]
                nc.tensor.matmul(
                    psum[:],
                    lhsT=x_tiles[pi][:, lc * B:(lc + 1) * B],
                    rhs=w_tiles[pi][:, lc * DIM:(lc + 1) * DIM],
                    start=(ck == 0),
                    stop=(ck == c - 1),
                )
            o_tile = out_pool.tile([B, DIM], bf16)
            nc.vector.tensor_copy(o_tile[:], psum[:])
            nc.scalar.dma_start(y_core[j], o_tile[:])

    nc.compile()
    return nc


def _run_per_core(ncs, in_maps):
    """Per-device execution of 8 distinct single-core programs (adapted from
    bass2jax.run_bass_via_pjrt's single-core path)."""
    from concurrent.futures import ThreadPoolExecutor

    from concourse import mybir as mb
    from concourse.bass2jax import _bass_exec_p, install_neuronx_cc_hook

    install_neuronx_cc_hook()
    devices = jax.devices()[:8]

    def launch(k):
        nc = ncs[k]
        in_names, out_names, out_avals, zero_outs = [], [], [], []
        for alloc in nc.m.functions[0].allocations:
            if not isinstance(alloc, mb.MemoryLocationSet):
                continue
            name = alloc.memorylocations[0].name
            if alloc.kind == "ExternalInput":
                in_names.append(name)
            elif alloc.kind == "ExternalOutput":
                shape = tuple(alloc.tensor_shape)
                dtype = mb.dt.np(alloc.dtype)
                out_names.append(name)
                out_avals.append(jax.core.ShapedArray(shape, dtype))
                zero_outs.append(np.zeros(shape, dtype))
        n_params = len(in_names)
        all_names = tuple(in_names + out_names)
        donate = tuple(range(n_params, n_params + len(out_names)))

        def _body(*args):
            outs = _bass_exec_p.bind(
                *args,
                out_avals=tuple(out_avals),
                in_names=all_names,
                out_names=tuple(out_names),
                lowering_input_output_aliases=(),
                sim_require_finite=True,
                sim_require_nnan=True,
                nc=nc,
            )
            return tuple(outs)

        dev = devices[k]
        extras = {}
        for alloc in nc.m.functions[0].allocations:
            if (isinstance(alloc, mb.MemoryLocationSet)
                    and alloc.kind == "ExternalInput"):
                name = alloc.memorylocations[0].name
                if name not in in_maps[k]:
                    extras[name] = np.full(
                        tuple(alloc.tensor_shape), k, mb.dt.np(alloc.dtype))
        args = [jax.device_put(np.asarray(in_maps[k].get(n, extras.get(n))), dev)
                for n in in_names]
        args += [jax.device_put(z, dev) for z in zero_outs]
        out_arrs = jax.jit(_body, donate_argnums=donate, keep_unused=True)(*args)
        return out_names, out_arrs

    with ThreadPoolExecutor(max_workers=8) as ex:
        futs = [ex.submit(launch, k) for k in range(8)]
        handles = [f.result() for f in futs]
    return [
        {name: np.asarray(arr) for name, arr in zip(names, arrs)}
        for names, arrs in handles
    ]


LAST_RESULTS = None


def kernel(x, W, bias, idx_a, idx_b, idx_c, lens):
    global LAST_RESULTS
    x = np.asarray(x, np.float32)
    W = np.asarray(W, np.float32)
    bias = np.asarray(bias, np.float32)
    idx_a = np.asarray(idx_a, np.int32)
    idx_b = np.asarray(idx_b, np.int32)
    idx_c = np.asarray(idx_c, np.int32)
    lens = np.asarray(lens, np.int32)
    assert x.shape == (B, CIN, IMG, IMG) and W.shape == (T, Lmax, DIM)

    in_maps, plans, tok_lists = _shard(x, W, bias, idx_a, idx_b, idx_c, lens)
    if "ncs" not in _cache:
        _cache["ncs"] = [_build_program(plans[k]) for k in range(NCORES)]
    ncs = _cache["ncs"]

    hook = None
    trace = os.environ.get("BASS_TRACE") and not os.environ.get("BASS_NEVER_TRACE")
    if trace:
        from antenv.axon_hooks import get_axon_ntff_profile_hook

        hook = get_axon_ntff_profile_hook()
    if hook is not None:
        tmpdir = os.environ.get("KERNEL_TRACE_TMPDIR") or "/tmp/kernel_trace"
        os.makedirs(tmpdir, exist_ok=True)
        with hook(tmpdir, [0]):
            results = _run_per_core(ncs, in_maps)
        LAST_RESULTS = ("ntff", tmpdir, ncs[0])
    else:
        results = _run_per_core(ncs, in_maps)
        LAST_RESULTS = None

    y = np.empty((B, T, DIM), np.float32)
    for k in range(NCORES):
        y[:, tok_lists[k], :] = results[k]["y_core"].transpose(1, 0, 2).astype(
            np.float32)
    return y
